# revision 59
# baseline (speedup 1.0000x reference)
"""Trainium2 Bass kernel for the CostVolume problem (self-contained).

Math (validated in numpy vs the jax reference, rel l2 ~1.33e-2 on device):
  conv1 of the shift-and-stack cost volume collapses into small 2D convs:
    - left half:  yL[h,w] (d-independent) + 4 diagonal variants at u=w-d in [-2,1]
    - right half: yR[h,u] on the (h, u=w-d) grid (mask == zero-padding there)
    - corrections: corr0 (d=0 plane), corr47 (d=47 plane), corrW (w=159 column)
  x1[d] planes are assembled with Pool/DVE adds + fused BN+ReLU activations.
  conv2 is a direct 3x3x3 conv over the x1 planes.

Precision/speed layout (231us -> 134.3us per core vs the all-f32r version):
  - conv1 runs entirely in bf16 (1 PE cycle/row at any alignment/size; the
    f32r even-offset constraint and small-N penalties disappear).
  - conv2 (the dominant cost) runs in fp8 e4m3 with DoubleRow perf mode
    (0.5 cycles/row, K=256 packed).  Accuracy is kept with a 3-term hi/lo
    split:  W^T x ~= Whi^T xhi + Wlo^T xhi + Whi^T xlo
    where Whi/Wlo are host-side e4m3 hi/residual pairs (x512 scale) and
    xhi/xlo are device-side e4m3 hi/residual pairs (x16 scale folded into
    bn1).  Each DoubleRow matmul packs the two row-pair block matmuls
    (L1, L2) of the 3x1 h-conv trick into its two K slots.
  - conv2 matmuls skip the constant region of the cost volume (x1[d] is the
    per-channel constant relu(c1) for w < d-2): plane q only computes
    w >= q-4 and the host-computed `stconst` plane fills the prefix
    (incrementally, 4 columns per plane, via the spool ring).

Per-plane pipeline (assembly issued 3 planes ahead of conv2):
  Pool: yL+yR main add -> DVE: dg/cw adds -> ACT: relu(bn1)x16 -> T8h (fp8,
  halo-row masks folded into per-partition scale/bias; 3 row-split calls)
  and in parallel DVE computes T2 = bn1-affine, then T8l = max(T2,0) - T8h.
  conv2's DoubleRow matmuls/plane order all xhi terms first, xlo last,
  so the T8l of plane q+1 is needed as late as possible; the two lowest-norm
  (kw, kd) tap-sets skip the xlo correction entirely (residual activation
  quantization error ~1.33e-2 rel l2, validated in numpy, vs the 2e-2 gate).

Downsample writes the row-pair layout directly: even rows via weights with
M at [0:64], odd rows via weights with M at [64:128], accumulating into one
PSUM tile; ACT evicts straight into Tlf/Trf (no shuffle DMA).  Input DMAs
are issued on the SP ring in need-order (the cost model's DMA lane is
serial); fp8/bf16 weights halve the stream vs f32.

Layout trick: row-pair interleaved partitions - partitions [0:64] hold the 64
channels of an even local row, [64:128] the following odd row; the free dim is
(row-pair, w).  A 3x1 conv in h needs TWO matmuls per output row pair, with
rhs = input pairs c and c+1 and block lhsT matrices L1=[[Ta,0],[Tb,Ta]],
L2=[[Tc,Tb],[0,Tc]]; in fp8 DoubleRow both land in one matmul.

Sharding: H-shard. Core k computes output rows [6k, 6k+6) from input rows
[6k-2, 6k+8) (zero-padded outside [0,48)).
"""
import os
import sys

sys.path.insert(0, "/opt/trn_rl_repo")

import ml_dtypes
import numpy as np

import concourse.bass as bass
import concourse.mybir as mybir
import concourse.tile as tile
from concourse import bacc
from concourse.bass_utils import run_bass_kernel_spmd

F32 = mybir.dt.float32
BF16 = mybir.dt.bfloat16
FP8 = mybir.dt.float8e4
AF = mybir.ActivationFunctionType
ALU = mybir.AluOpType
PM = mybir.MatmulPerfMode

H, W, DEPTH, PSM, CIN = 48, 160, 48, 64, 256
NC = 8
HS = H // NC          # 6 output rows per core
RIN = HS + 4          # 10 input rows per core
NPI = RIN // 2        # 5 input row pairs
WP = 168              # lf/rf row width, col = w + 4  (w in [-4, 163])
WT = 162              # x1/yL/corr row width, col = w + 1 (w in [-1, 160])
U0 = 50               # yR col = u + U0, u in [-U0, 160)
WU = U0 + W           # 210
BN_EPS = 1e-3
WSC = 512.0           # conv2 weight fp8 scale
XSC = 16.0            # conv2 activation fp8 scale (folded into bn1)

_cache = {}


# ---------------------------------------------------------------- host prep --
def _bn_fold(g, b, m, v, conv_bias):
    a = (g / np.sqrt(v + BN_EPS)).astype(np.float32)
    c = (b + (conv_bias - m) * a).astype(np.float32)
    return a, c


def _q8(x):
    return x.astype(ml_dtypes.float8_e4m3).astype(np.float32)


def _prep_weights(inputs):
    """Returns (wtb bf16 [128, N1*128], wt8 fp8 [128, N8*128], idx maps,
    consts [128,8], bn-fold vectors)."""
    c1_w = np.asarray(inputs["c1_w"], np.float32)
    c2_w = np.asarray(inputs["c2_w"], np.float32)
    ds_w = np.asarray(inputs["ds_w"], np.float32)
    W1L = c1_w[:, :, :, :PSM, :]   # [kh, kw, kd, 64, 64]
    W1R = c1_w[:, :, :, PSM:, :]

    Z = np.zeros((PSM, PSM), np.float32)

    def L1(Ta, Tb):  # rhs pair c:  half0 += Ta^T x_ev + Tb^T x_od ; half1 += Ta^T x_od
        return np.block([[Ta, Z], [Tb, Ta]])

    def L2(Tb, Tc):  # rhs pair c+1: half0 += Tc^T x_ev ; half1 += Tb^T x_ev + Tc^T x_od
        return np.block([[Tc, Tb], [Z, Tc]])

    slots, idx = [], {}

    def add(name, mat):
        assert mat.shape == (128, 128), (name, mat.shape)
        idx[name] = len(slots)
        slots.append(mat.astype(np.float32))

    def add_pair(base, T3):  # T3[kh] for kh=0,1,2
        add(base + "_1", L1(T3[0], T3[1]))
        add(base + "_2", L2(T3[1], T3[2]))

    # downsample: K=256 split into two 128-halves; M placed at [0:64] for
    # even rows and [64:128] for odd rows so the matmuls write the row-pair
    # interleaved layout directly (no shuffle DMA).
    Z64 = np.zeros((128, 64), np.float32)
    add("ds0", np.concatenate([ds_w[:128], Z64], axis=1))
    add("ds1", np.concatenate([ds_w[128:], Z64], axis=1))
    add("ds0o", np.concatenate([Z64, ds_w[:128]], axis=1))
    add("ds1o", np.concatenate([Z64, ds_w[128:]], axis=1))

    # yL full: sum over all kd
    TF = W1L.sum(axis=2)  # [kh, kw, 64, 64]
    for kw in range(3):
        add_pair(f"yl_{kw}", TF[:, kw])

    # yR: V[kh, s+2] = sum_{(kw-1)-(kd-1)=s} W1R[kh,kw,kd]
    V = np.zeros((3, 5, PSM, PSM), np.float32)
    for kw in range(3):
        for kd in range(3):
            V[:, (kw - kd) + 2] += W1R[:, kw, kd]
    for si in range(5):
        add_pair(f"yr_{si}", V[:, si])

    # diagonal yL variants u in {-2,-1,0,1}: sum over kd with (kd-1) <= (kw-1)+u
    for ui, u in enumerate((-2, -1, 0, 1)):
        TU = np.zeros((3, 3, PSM, PSM), np.float32)
        for kw in range(3):
            for kd in range(3):
                if (kd - 1) <= (kw - 1) + u:
                    TU[:, kw] += W1L[:, kw, kd]
        for kw in range(3):
            add_pair(f"dg{ui}_{kw}", TU[:, kw])

    # corr0 (kd=0 plane read at d=-1): lf taps + rf taps (rf read at w+kw)
    for kw in range(3):
        add_pair(f"c0l_{kw}", W1L[:, kw, 0])
        add_pair(f"c0r_{kw}", W1R[:, kw, 0])

    # corr47 (kd=2 plane read at d=48, masked w+dw>=48)
    for kw in range(3):
        add_pair(f"c47l_{kw}", W1L[:, kw, 2])
        add_pair(f"c47r_{kw}", W1R[:, kw, 2])

    # corrW (w=159 column fix), reversed index t = 47-d
    for di in range(3):
        add_pair(f"cw{di}", W1R[:, 2, di])

    wtb = np.concatenate(slots, axis=1).astype(ml_dtypes.bfloat16)

    # conv2 fp8 hi/lo DoubleRow slot pairs: per (kd, kw) 4 slots:
    # [hi_1, hi_2, lo_1, lo_2]  (each 128x128, e4m3 of 512*L +/- residual)
    slots8, idx8 = [], {}
    for kd in range(3):
        for kw in range(3):
            l1 = L1(c2_w[0, kw, kd], c2_w[1, kw, kd]) * WSC
            l2 = L2(c2_w[1, kw, kd], c2_w[2, kw, kd]) * WSC
            h1, h2 = _q8(l1), _q8(l2)
            idx8[f"c2_{kw}{kd}"] = len(slots8)
            slots8 += [h1, h2, l1 - h1, l2 - h2]
    wt8 = np.concatenate(slots8, axis=1).astype(ml_dtypes.float8_e4m3)

    a0, c0 = _bn_fold(*[np.asarray(inputs[f"bn0_{x}"], np.float32) for x in "gbmv"],
                      np.asarray(inputs["ds_b"], np.float32))
    a1, c1 = _bn_fold(*[np.asarray(inputs[f"bn1_{x}"], np.float32) for x in "gbmv"],
                      np.asarray(inputs["c1_b"], np.float32))
    a2, c2 = _bn_fold(*[np.asarray(inputs[f"bn2_{x}"], np.float32) for x in "gbmv"],
                      np.asarray(inputs["c2_b"], np.float32))
    x1c16 = XSC * np.maximum(c1, 0.0)
    x1c16_res = x1c16 - _q8(x1c16)   # fp8 residual of the const-region value
    consts = np.zeros((128, 8), np.float32)
    for j, v in enumerate((a0, c0, XSC * a1, XSC * c1, a2 / (WSC * XSC), c2,
                           x1c16, x1c16_res)):
        consts[:, j] = np.tile(v, 2)
    return wtb, wt8, idx, idx8, consts, (a1, c1, a2, c2, a0, c0)


def _stconst(core, c2_w, bnf):
    """Constant-region conv2 output planes [2, 128, 3, W] for this core.

    x1[d] for w < d-2 is the per-channel constant relu(c1); conv2 of that
    constant (with h/w/d zero-padding at the volume borders) is a per-channel,
    per-row-variant, per-col-variant constant.  Variant 0: interior planes
    q in [5, 46] (all kd taps); variant 1: plane 47 (kd=2 reads zero-pad)."""
    a1, c1, a2, c2 = bnf[:4]
    x1c = np.maximum(c1, 0.0)
    out = np.zeros((2, 2, 64, 3, W), np.float32)  # [var, half, ch, pair, w]
    for var, kds in ((0, (0, 1, 2)), (1, (0, 1))):
        s = np.einsum("hwdcm,c->hwm", c2_w[:, :, kds], x1c)  # [kh, kw, m]
        for pair in range(3):
            for half in range(2):
                g = 6 * core + 2 * pair + half    # global output row
                khs = [kh for kh in range(3) if 0 <= g + kh - 1 < H]
                for w in (0, 1):                  # col variant: w==0 misses kw=0
                    kws = [kw for kw in range(3) if w + kw - 1 >= 0]
                    z = s[np.ix_(khs, kws)].sum(axis=(0, 1))
                    col = np.maximum(a2 * z + c2, 0.0)
                    if w == 0:
                        out[var, half, :, pair, 0] = col
                    else:
                        out[var, half, :, pair, 1:] = col[:, None]
    return out.reshape(2, 128, 3, W)


def _prep_core_inputs(inputs, wtb, wt8, consts, bnf):
    lfull = np.asarray(inputs["left_features"], np.float32)[0]
    rfull = np.asarray(inputs["right_features"], np.float32)[0]
    c2_w = np.asarray(inputs["c2_w"], np.float32)
    a1, c1, bn0a, bn0c = bnf[0], bnf[1], bnf[4], bnf[5]
    a1t, c1t = np.tile(a1, 2), np.tile(c1, 2)
    in_maps = []
    for k in range(NC):
        feats = np.zeros((2, RIN, W, CIN), np.float32)
        g0 = 6 * k - 2
        lo, hi = max(0, g0), min(H, g0 + RIN)
        if hi > lo:
            feats[0, lo - g0:hi - g0] = lfull[lo:hi]
            feats[1, lo - g0:hi - g0] = rfull[lo:hi]
        # -> [128, 2(lr), 2(khalf), RIN, W]
        ft = feats.transpose(0, 3, 1, 2).reshape(2, 2, 128, RIN, W).transpose(2, 0, 1, 3, 4)
        aux = np.zeros((128, 12), np.float32)
        a0t, c0t_ = np.tile(bn0a, 2), np.tile(bn0c, 2)
        mp0 = np.ones(128, np.float32)                 # feats pair 0 (rows 6k-2,-1)
        mp0[:64] = 1.0 if g0 >= 0 else 0.0
        mp0[64:] = 1.0 if g0 + 1 >= 0 else 0.0
        mp4 = np.ones(128, np.float32)                 # feats pair 4 (rows 6k+6,+7)
        mp4[:64] = 1.0 if g0 + 8 < H else 0.0
        mp4[64:] = 1.0 if g0 + 9 < H else 0.0
        aux[:, 0] = a0t * mp0
        aux[:, 1] = c0t_ * mp0
        aux[:, 2] = a0t * mp4
        aux[:, 3] = c0t_ * mp4
        m0 = np.ones(128, np.float32)                  # x1 row 0 (global 6k-1)
        if 6 * k - 1 < 0:
            m0[:64] = 0.0
        m3 = np.ones(128, np.float32)                  # x1 row 7 (global 6k+6)
        if 6 * k + 6 >= H:
            m3[64:] = 0.0
        aux[:, 7] = XSC * a1t * m0
        aux[:, 8] = XSC * c1t * m0
        aux[:, 9] = XSC * a1t * m3
        aux[:, 10] = XSC * c1t * m3
        in_maps.append({
            "feats": np.ascontiguousarray(ft).astype(ml_dtypes.bfloat16),
            "wtb": wtb,
            "wt8": wt8,
            "cstaux": np.concatenate([consts, aux], axis=1),
            "stconst": _stconst(k, c2_w, bnf),
        })
    return in_maps


# ------------------------------------------------------------- bass program --
def _build_program(idx, idx8, debug=False):
    nc = bacc.Bacc()
    N1 = len(idx)

    feats_d = nc.declare_dram_parameter("feats", [128, 2, 2, RIN, W], BF16, isOutput=False)
    wtb_d = nc.declare_dram_parameter("wtb", [128, N1 * 128], BF16, isOutput=False)
    wt8_d = nc.declare_dram_parameter("wt8", [128, 36 * 128], FP8, isOutput=False)
    cstaux_d = nc.declare_dram_parameter("cstaux", [128, 20], F32, isOutput=False)
    stc_d = nc.declare_dram_parameter("stconst", [2, 128, 3, W], F32, isOutput=False)
    out_d = nc.declare_dram_parameter("out", [DEPTH, 128, 3 * W], F32, isOutput=True)
    dbg = {}
    if debug:
        for name, shape in (("dbg_tlf", [128, NPI, WP]), ("dbg_trf", [128, NPI, WP]),
                            ("dbg_yl", [128, 4, WT]), ("dbg_yr", [128, 4, WU]),
                            ("dbg_x1", [DEPTH, 128, 4, WT])):
            dbg[name] = nc.declare_dram_parameter(name, shape, F32, isOutput=True)

    with tile.TileContext(nc) as tc, (
        tc.tile_pool(name="cpool", bufs=1)
    ) as cpool, tc.tile_pool(
        name="spool", bufs=4
    ) as spool, tc.tile_pool(
        name="psds", bufs=2, space="PSUM"
    ) as psds_pool, tc.tile_pool(
        name="psc1", bufs=2, space="PSUM"
    ) as psc1_pool, tc.tile_pool(name="psc2", bufs=4, space="PSUM") as psc2_pool:

        wtb = cpool.tile([128, N1 * 128], BF16, tag="wtb")
        wt8 = cpool.tile([128, 36 * 128], FP8, tag="wt8")
        cstaux = cpool.tile([128, 20], F32, tag="cstaux")
        cst = cstaux[:, 0:8]
        aux = cstaux[:, 8:20]
        stc = cpool.tile([128, 2, 3, W], F32, tag="stc")
        FtL = cpool.tile([128, 2, RIN, W], BF16, tag="FtL")
        FtR = cpool.tile([128, 2, RIN, W], BF16, tag="FtR")
        Tlf = cpool.tile([128, NPI, WP], BF16, tag="Tlf")
        Trf = cpool.tile([128, NPI, WP], BF16, tag="Trf")
        yL = cpool.tile([128, 4, WT], F32, tag="yL")
        yR = cpool.tile([128, 4, WU], F32, tag="yR")
        dg = cpool.tile([128, 4, DEPTH, 4], F32, tag="dg")
        cw = cpool.tile([128, 4, DEPTH], F32, tag="cw")
        c0t = cpool.tile([128, 4, WT], F32, tag="c0t")
        c47t = cpool.tile([128, 4, WT], F32, tag="c47t")
        # x1 staging: rotating f32 tiles (pads zeroed once, never rewritten)
        Tx = [cpool.tile([128, 4, WT], F32, tag=f"Tx{i}", name=f"Tx{i}")
              for i in range(4)]
        T2x = [cpool.tile([128, 4, WT], F32, tag=f"T2x{i}", name=f"T2x{i}")
               for i in range(3)]
        # fp8 hi/lo rings as explicit tiles: pad cols 0/161 memset once and
        # never rewritten (all per-plane ops stay within [1, 161))
        T8H = [cpool.tile([128, 4, WT], FP8, tag=f"T8H{i}", name=f"T8H{i}")
               for i in range(6)]
        T8L = [cpool.tile([128, 4, WT], FP8, tag=f"T8L{i}", name=f"T8L{i}")
               for i in range(6)]

        # Stream weight/feature loads in per-group chunks, issued in program
        # order, so each matmul group starts as soon as its slots land.
        # Slot layout: ds 0-3, yl 4-9, yr 10-19, dg 20-43, c0 44-55,
        # c47 56-67, cw 68-73.  Late chunks (c0/c47/wt8/stc) are issued from
        # the ACT ring after the ds section so their transfers don't block
        # the early ones on the serial DMA lane.
        nc.sync.dma_start(wtb[:, :4 * 128], wtb_d[:, :4 * 128])
        nc.sync.dma_start(FtR[:, :, 0:6], feats_d[:, 1, :, 0:6])
        nc.sync.dma_start(cstaux[:], cstaux_d[:])
        nc.sync.dma_start(FtR[:, :, 6:RIN], feats_d[:, 1, :, 6:RIN])
        nc.sync.dma_start(wtb[:, 10 * 128:20 * 128], wtb_d[:, 10 * 128:20 * 128])
        nc.sync.dma_start(FtL[:], feats_d[:, 0])
        nc.sync.dma_start(wtb[:, 4 * 128:10 * 128], wtb_d[:, 4 * 128:10 * 128])
        nc.sync.dma_start(wtb[:, 20 * 128:44 * 128], wtb_d[:, 20 * 128:44 * 128])
        nc.sync.dma_start(wtb[:, 68 * 128:74 * 128], wtb_d[:, 68 * 128:74 * 128])
        nc.sync.dma_start(wtb[:, 44 * 128:56 * 128], wtb_d[:, 44 * 128:56 * 128])
        nc.sync.dma_start(wtb[:, 56 * 128:68 * 128], wtb_d[:, 56 * 128:68 * 128])
        nc.sync.dma_start(wt8[:], wt8_d[:])
        nc.sync.dma_start(stc[:, 0], stc_d[0])
        nc.sync.dma_start(stc[:, 1], stc_d[1])

        nc.gpsimd.memset(Tlf[:].bitcast(mybir.dt.uint16), 0)
        nc.gpsimd.memset(Trf[:].bitcast(mybir.dt.uint16), 0)
        nc.gpsimd.memset(yR[:], 0.0)
        for t in Tx:
            nc.gpsimd.memset(t[:], 0.0)
        for t in T8H + T8L:
            nc.gpsimd.memset(t[:].bitcast(mybir.dt.uint8), 0)

        def ws(name):
            i = idx[name]
            return wtb[:, i * 128:(i + 1) * 128]

        def emit(ps_ap, mms):
            for i, (name, rhs) in enumerate(mms):
                nc.tensor.matmul(ps_ap, ws(name), rhs,
                                 start=(i == 0), stop=(i == len(mms) - 1))

        # consolidate DMA waits into dummy matmuls so compute matmuls only
        # ever need one new wait (their rhs producer).
        def emit_dummy(slot):
            ps_d = psds_pool.tile([128, 8], F32, tag="ps")
            nc.tensor.matmul(ps_d[:], wtb[:, slot * 128:slot * 128 + 128],
                             wtb[:, 0:8], start=True, stop=True)

        emit_dummy(0)

        # ---- downsample: lf/rf = relu(bn0(feats @ ds_w + ds_b)) -------------
        # even rows via ds0/ds1 (M at [0:64]), odd rows via ds0o/ds1o (M at
        # [64:128]) accumulate into one PSUM tile already in pair layout;
        # ACT evicts straight into Tlf/Trf (no shuffle DMA).
        def ds_half(ft, dst):
            f2 = ft.rearrange("p k (r two) w -> p k r two w", two=2)
            for p0, npair in ((0, 3), (3, 2)):
                ps = psds_pool.tile([128, 3, W], F32, tag="ps")
                mmds = [(f"ds{kk}", f2[:, kk, p0:p0 + npair, 0, :])
                        for kk in (0, 1)]
                mmds += [(f"ds{kk}o", f2[:, kk, p0:p0 + npair, 1, :])
                         for kk in (0, 1)]
                for i, (nm, rhs) in enumerate(mmds):
                    nc.tensor.matmul(ps[:, :npair], ws(nm), rhs,
                                     start=(i == 0), stop=(i == len(mmds) - 1))
                if p0 == 0:   # pair 0 carries the top halo mask (core 0)
                    nc.scalar.activation(dst[:, 0, 4:4 + W], ps[:, 0],
                                         AF.Relu, bias=aux[:, 1:2], scale=aux[:, 0:1])
                    nc.scalar.activation(dst[:, 1:3, 4:4 + W], ps[:, 1:3],
                                         AF.Relu, bias=cst[:, 1:2], scale=cst[:, 0:1])
                else:         # pair 4 carries the bottom halo mask (core 7)
                    nc.scalar.activation(dst[:, 3, 4:4 + W], ps[:, 0],
                                         AF.Relu, bias=cst[:, 1:2], scale=cst[:, 0:1])
                    nc.scalar.activation(dst[:, 4, 4:4 + W], ps[:, 1],
                                         AF.Relu, bias=aux[:, 3:4], scale=aux[:, 2:3])

        ds_half(FtR, Trf)
        emit_dummy(10)  # yr chunk
        if debug:
            nc.sync.dma_start(dbg["dbg_tlf"][:], Tlf[:])
            nc.sync.dma_start(dbg["dbg_trf"][:], Trf[:])

        # ---- yR on the (h, u) grid (Trf lands first) ------------------------
        for hf in (0, 1):
            ps = psc1_pool.tile([128, 2, WT], F32, tag="ps")
            mms = []
            for si in range(5):
                mms.append((f"yr_{si}_1", Trf[:, 2 * hf:2 * hf + 2, si:si + WT]))
                mms.append((f"yr_{si}_2", Trf[:, 2 * hf + 1:2 * hf + 3, si:si + WT]))
            emit(ps[:], mms)
            nc.vector.tensor_copy(yR[:, 2 * hf:2 * hf + 2, U0 - 2:U0 - 2 + WT], ps[:])

        ds_half(FtL, Tlf)
        emit_dummy(4)   # yl chunk

        # ---- yL (d-independent left conv) -----------------------------------
        for hf in (0, 1):
            ps = psc1_pool.tile([128, 2, W], F32, tag="ps")
            mms = []
            for kw in range(3):
                mms.append((f"yl_{kw}_1", Tlf[:, 2 * hf:2 * hf + 2, kw + 3:kw + 3 + W]))
                mms.append((f"yl_{kw}_2", Tlf[:, 2 * hf + 1:2 * hf + 3, kw + 3:kw + 3 + W]))
            emit(ps[:], mms)
            nc.vector.tensor_copy(yL[:, 2 * hf:2 * hf + 2, 1:1 + W], ps[:])

        # ---- diagonal yL variants (evaluated at w = d+u, d = 0..47) ---------
        emit_dummy(20)  # dg chunk
        for ui in range(4):
            u = ui - 2
            ps = psc1_pool.tile([128, 4, DEPTH], F32, tag="ps")
            mms = []
            for kw in range(3):
                s0 = u + kw + 3
                mms.append((f"dg{ui}_{kw}_1", Tlf[:, 0:4, s0:s0 + DEPTH]))
                mms.append((f"dg{ui}_{kw}_2", Tlf[:, 1:5, s0:s0 + DEPTH]))
            emit(ps[:], mms)
            nc.vector.tensor_copy(dg[:, :, :, ui], ps[:])

        # ---- corrW (w=159 column), reversed index t = 47-d ------------------
        emit_dummy(68)  # cw chunk
        ps_cw = psc1_pool.tile([128, 4, DEPTH], F32, tag="ps")
        mms = []
        for di in range(3):
            s0 = 117 - (di - 1)
            mms.append((f"cw{di}_1", Trf[:, 0:4, s0:s0 + DEPTH]))
            mms.append((f"cw{di}_2", Trf[:, 1:5, s0:s0 + DEPTH]))
        emit(ps_cw[:], mms)
        nc.vector.tensor_copy(cw[:], ps_cw[:])

        # ---- corr0 ----------------------------------------------------------
        def c0_group():
            emit_dummy(44)  # c0 chunk
            for hf in (0, 1):
                ps = psc1_pool.tile([128, 2, W], F32, tag="ps")
                mms = []
                for kw in range(3):
                    mms.append((f"c0l_{kw}_1", Tlf[:, 2 * hf:2 * hf + 2, kw + 3:kw + 3 + W]))
                    mms.append((f"c0l_{kw}_2", Tlf[:, 2 * hf + 1:2 * hf + 3, kw + 3:kw + 3 + W]))
                    mms.append((f"c0r_{kw}_1", Trf[:, 2 * hf:2 * hf + 2, kw + 4:kw + 4 + W]))
                    mms.append((f"c0r_{kw}_2", Trf[:, 2 * hf + 1:2 * hf + 3, kw + 4:kw + 4 + W]))
                emit(ps[:], mms)
                nc.vector.tensor_copy(c0t[:, 2 * hf:2 * hf + 2, 1:1 + W], ps[:])

        # ---- corr47 (valid only for w >= 47; per-kw masked column ranges) ---
        # runs after conv2(0): only assembly(47) consumes it
        def c47_group():
            emit_dummy(56)  # c47 chunk
            for hf in (0, 1):
                ps = psc1_pool.tile([128, 2, W], F32, tag="ps")
                mm_list = []
                for kw in (2, 1, 0):
                    w0 = 49 - kw
                    nw = W - w0
                    for base, src, rs0 in ((f"c47l_{kw}", Tlf, 52),
                                           (f"c47r_{kw}", Trf, 4)):
                        # rf part at kw=2 must not cover w=159 (corrW's job)
                        nw_eff = nw - 1 if (kw == 2 and src is Trf) else nw
                        for sfx, p0 in (("_1", 0), ("_2", 1)):
                            for cc in (0, 1):
                                mm_list.append((base + sfx, cc, w0, nw_eff, src, p0, rs0))
                for i, (nm, cc, w0, nw, src, p0, rs0) in enumerate(mm_list):
                    nc.tensor.matmul(
                        ps[:, cc, w0:w0 + nw], ws(nm),
                        src[:, 2 * hf + p0 + cc, rs0:rs0 + nw],
                        start=(i == 0), stop=(i == len(mm_list) - 1))
                nc.vector.tensor_copy(c47t[:, 2 * hf:2 * hf + 2, 48:1 + W],
                                      ps[:, :, 47:W])

        if debug:
            nc.sync.dma_start(dbg["dbg_yl"][:], yL[:])
            nc.sync.dma_start(dbg["dbg_yr"][:], yR[:])

        # fp8 weights dummy (consume the wt8 DMA wait once)
        ps_d8 = psds_pool.tile([128, 8], F32, tag="ps")
        nc.tensor.matmul(ps_d8[:], wt8[:, 0:128], wt8[:, 0:8], start=True, stop=True)

        # ---- x1 plane assembly + conv2 --------------------------------------
        t8h, t8l = {}, {}

        def assembly(d):
            T = Tx[d % 4]
            T2 = T2x[d % 3]
            T8h = T8H[d % 6]
            T8l = T8L[d % 6]
            t8h[d], t8l[d] = T8h, T8l
            # conv2 only ever reads this plane's cols >= d-5; trim left edges
            sa = max(1, d - 6)
            s8 = max(1, d - 5)
            # raw conv1 sums (main add on Pool to unload DVE)
            nc.gpsimd.tensor_tensor(T[:, :, sa:1 + W], yL[:, :, sa:1 + W],
                                    yR[:, :, U0 - d + sa - 1:U0 - d + W], ALU.add)
            wlo = max(0, d - 2)
            ncol = d + 2 - wlo
            nc.vector.tensor_tensor(T[:, :, 1 + wlo:1 + wlo + ncol],
                                    dg[:, :, d, 4 - ncol:4],
                                    yR[:, :, U0 + wlo - d:U0 + 2], ALU.add)
            nc.vector.tensor_tensor(T[:, :, W:W + 1], T[:, :, W:W + 1],
                                    cw[:, :, 47 - d:48 - d], ALU.subtract)
            if d == 0:
                nc.vector.tensor_tensor(T[:, :, 1:1 + W], T[:, :, 1:1 + W],
                                        c0t[:, :, 1:1 + W], ALU.subtract)
            if d == DEPTH - 1:
                nc.vector.tensor_tensor(T[:, :, 48:1 + W], T[:, :, 48:1 + W],
                                        c47t[:, :, 48:1 + W], ALU.subtract)
            # hi: ACT writes relu(bn1*16) straight to fp8, halo-row masks
            # folded into per-partition scale/bias for pairs 0 and 3
            nc.scalar.activation(T8h[:, 1:3, sa:1 + W], T[:, 1:3, sa:1 + W],
                                 AF.Relu, bias=cst[:, 3:4], scale=cst[:, 2:3])
            nc.scalar.activation(T8h[:, 0, sa:1 + W], T[:, 0, sa:1 + W],
                                 AF.Relu, bias=aux[:, 8:9], scale=aux[:, 7:8])
            nc.scalar.activation(T8h[:, 3, sa:1 + W], T[:, 3, sa:1 + W],
                                 AF.Relu, bias=aux[:, 10:11], scale=aux[:, 9:10])
            # lo: bn-affine on DVE (independent of ACT), then
            # T8l = max(T2, 0) - T8h once the ACT hi is out
            nc.vector.tensor_scalar(T2[:, 1:3, s8:1 + W], T[:, 1:3, s8:1 + W],
                                    cst[:, 2:3], cst[:, 3:4], ALU.mult, ALU.add)
            nc.vector.tensor_scalar(T2[:, 0, s8:1 + W], T[:, 0, s8:1 + W],
                                    aux[:, 7:8], aux[:, 8:9], ALU.mult, ALU.add)
            nc.vector.tensor_scalar(T2[:, 3, s8:1 + W], T[:, 3, s8:1 + W],
                                    aux[:, 9:10], aux[:, 10:11], ALU.mult, ALU.add)
            nc.vector.scalar_tensor_tensor(T8l[:, :, s8:1 + W], T2[:, :, s8:1 + W],
                                           0.0, T8h[:, :, s8:1 + W],
                                           ALU.max, ALU.subtract)
            # const region: only the tail read by conv2's halo needs to be
            # correct (w >= d-7); the rest is covered by the stconst prefix
            if d >= 3:
                s0 = max(1, d - 7)
                nc.scalar.activation(T8h[:, :, s0:d - 1], T8h[:, :, s0:d - 1],
                                     AF.Identity, bias=cst[:, 6:7], scale=0.0)
                nc.scalar.activation(T8l[:, :, s0:d - 1], T8l[:, :, s0:d - 1],
                                     AF.Identity, bias=cst[:, 7:8], scale=0.0)
            if debug:
                nc.sync.dma_start(dbg["dbg_x1"][d], T[:])

        def dr_rhs(t8, kw, c0_, nw):
            # [128, 2(L1/L2 pair base), 3(out pair), nw] overlapping window
            base = t8[:, 0, 0]
            return bass.AP(
                base.tensor, base.offset + c0_ + kw,
                [list(base.ap[0]), [WT, 2], [WT, 3], [1, nw]])

        def dr_lhs(islot):
            return wt8[:, islot * 128:(islot + 2) * 128].rearrange(
                "p (two m) -> p two m", two=2)

        def conv2(q):
            wq0 = max(0, q - 4)
            ps = psc2_pool.tile([128, 3, W], F32, tag="ps")
            kds = (2, 1, 0) if q == 0 else (0, 1, 2)
            mms = []
            for kd in kds:
                p = q + kd - 1
                if p < 0 or p >= DEPTH:
                    continue
                for kw in range(3):
                    s = idx8[f"c2_{kw}{kd}"]
                    mms.append((s, t8h[p], kw))      # hi * xhi
                    mms.append((s + 2, t8h[p], kw))  # lo * xhi
            for kd in kds:                           # xlo terms last: T8l of
                p = q + kd - 1                       # the freshest plane lands
                if p < 0 or p >= DEPTH:              # latest
                    continue
                for kw in range(3):
                    # the two lowest-norm tap-sets skip the xlo correction:
                    # residual act-quant error ~1.3e-2 rel l2 (validated in
                    # numpy), saving 2/9 of the xlo matmul volume
                    if kd == 2 and kw in (0, 2):
                        continue
                    mms.append((idx8[f"c2_{kw}{kd}"], t8l[p], kw))
            # w-chunks of <=80 so DoubleRow rhs free stays <=512
            chunks = []
            c = wq0
            while c < W:
                nw = min(80, W - c)
                chunks.append((c, nw))
                c += nw
            n = len(mms) * len(chunks)
            i = 0
            for (islot, t8, kw) in mms:
                for (c0_, nw) in chunks:
                    nc.tensor.matmul(
                        ps[:, :, c0_:c0_ + nw], dr_lhs(islot),
                        dr_rhs(t8, kw, c0_, nw),
                        start=(i == 0), stop=(i == n - 1),
                        perf_mode=PM.DoubleRow)
                    i += 1
            st = spool.tile([128, 3, W], F32, tag="st")
            # bn2+relu on ACT (Pool cannot read PSUM).  The stconst prefix is
            # incremental: this spool buffer already holds stc[0][0:wq0(q-4)]
            # from plane q-4, so only [wq0(q-4), wq0) needs writing -- except
            # plane 47, whose const differs (missing kd=2 tap).
            nc.scalar.activation(st[:, :, wq0:], ps[:, :, wq0:], AF.Relu,
                                 bias=cst[:, 5:6], scale=cst[:, 4:5])
            if q == DEPTH - 1:
                nc.vector.tensor_copy(st[:, :, 0:wq0], stc[:, 1, :, 0:wq0])
            elif wq0 > 0:
                pw0 = max(0, q - 8)
                nc.vector.tensor_copy(st[:, :, pw0:wq0], stc[:, 0, :, pw0:wq0])
            nc.sync.dma_start(out_d[q].rearrange("p (c w) -> p c w", c=3), st[:])

        assembly(1)
        assembly(2)
        c0_group()
        assembly(0)
        c47_group()
        for q in range(DEPTH):
            conv2(q)
            if q + 3 < DEPTH:
                assembly(q + 3)
            t8h.pop(q - 1, None)
            t8l.pop(q - 1, None)

    nc.compile()
    return nc


def kernel(**inputs):
    wtb, wt8, idx, idx8, consts, bnf = _prep_weights(inputs)
    if "nc" not in _cache:
        _cache["nc"] = _build_program(idx, idx8)
    nc = _cache["nc"]
    in_maps = _prep_core_inputs(inputs, wtb, wt8, consts, bnf)
    trace = os.environ.get("COSTVOL_TRACE") == "1"
    res = run_bass_kernel_spmd(nc, in_maps, list(range(NC)), trace=trace)
    _cache["exec_time_ns"] = res.exec_time_ns
    out = np.zeros((H, W, DEPTH * PSM), np.float32)
    for k in range(NC):
        r = np.asarray(res.results[k]["out"])            # [48, 128, 480]
        blk = (r.reshape(DEPTH, 2, PSM, 3, W)
                .transpose(3, 1, 4, 0, 2)
                .reshape(HS, W, DEPTH * PSM))
        out[6 * k:6 * k + HS] = blk
    return out[None]


# revision 60
# speedup vs baseline: 1.0318x; 1.0318x over previous
"""Trainium2 Bass kernel for the CostVolume problem (self-contained).

Math (validated in numpy vs the jax reference, rel l2 ~1.33e-2 on device):
  conv1 of the shift-and-stack cost volume collapses into small 2D convs:
    - left half:  yL[h,w] (d-independent) + 4 diagonal variants at u=w-d in [-2,1]
    - right half: yR[h,u] on the (h, u=w-d) grid (mask == zero-padding there)
    - corrections: corr0 (d=0 plane), corr47 (d=47 plane), corrW (w=159 column)
  x1[d] planes are assembled with Pool/DVE adds + fused BN+ReLU activations.
  conv2 is a direct 3x3x3 conv over the x1 planes.

Precision/speed layout (231us -> 134.3us per core vs the all-f32r version):
  - conv1 runs entirely in bf16 (1 PE cycle/row at any alignment/size; the
    f32r even-offset constraint and small-N penalties disappear).
  - conv2 (the dominant cost) runs in fp8 e4m3 with DoubleRow perf mode
    (0.5 cycles/row, K=256 packed).  Accuracy is kept with a 3-term hi/lo
    split:  W^T x ~= Whi^T xhi + Wlo^T xhi + Whi^T xlo
    where Whi/Wlo are host-side e4m3 hi/residual pairs (x512 scale) and
    xhi/xlo are device-side e4m3 hi/residual pairs (x16 scale folded into
    bn1).  Each DoubleRow matmul packs the two row-pair block matmuls
    (L1, L2) of the 3x1 h-conv trick into its two K slots.
  - conv2 matmuls skip the constant region of the cost volume (x1[d] is the
    per-channel constant relu(c1) for w < d-2): plane q only computes
    w >= q-4 and the host-computed `stconst` plane fills the prefix
    (incrementally, 4 columns per plane, via the spool ring).

Per-plane pipeline (assembly issued 3 planes ahead of conv2):
  Pool: yL+yR main add -> DVE: dg/cw adds -> ACT: relu(bn1)x16 -> T8h (fp8,
  halo-row masks folded into per-partition scale/bias; 3 row-split calls)
  and in parallel DVE computes T2 = bn1-affine, then T8l = max(T2,0) - T8h.
  conv2's DoubleRow matmuls/plane order all xhi terms first, xlo last,
  so the T8l of plane q+1 is needed as late as possible; the two lowest-norm
  (kw, kd) tap-sets skip the xlo correction entirely (residual activation
  quantization error ~1.33e-2 rel l2, validated in numpy, vs the 2e-2 gate).

Downsample writes the row-pair layout directly: even rows via weights with
M at [0:64], odd rows via weights with M at [64:128], accumulating into one
PSUM tile; ACT evicts straight into Tlf/Trf (no shuffle DMA).  Input DMAs
are issued on the SP ring in need-order (the cost model's DMA lane is
serial); fp8/bf16 weights halve the stream vs f32.

Layout trick: row-pair interleaved partitions - partitions [0:64] hold the 64
channels of an even local row, [64:128] the following odd row; the free dim is
(row-pair, w).  A 3x1 conv in h needs TWO matmuls per output row pair, with
rhs = input pairs c and c+1 and block lhsT matrices L1=[[Ta,0],[Tb,Ta]],
L2=[[Tc,Tb],[0,Tc]]; in fp8 DoubleRow both land in one matmul.

Sharding: H-shard. Core k computes output rows [6k, 6k+6) from input rows
[6k-2, 6k+8) (zero-padded outside [0,48)).
"""
import os
import sys

sys.path.insert(0, "/opt/trn_rl_repo")

import ml_dtypes
import numpy as np

import concourse.bass as bass
import concourse.mybir as mybir
import concourse.tile as tile
from concourse import bacc
from concourse.bass_utils import run_bass_kernel_spmd

F32 = mybir.dt.float32
BF16 = mybir.dt.bfloat16
FP8 = mybir.dt.float8e4
AF = mybir.ActivationFunctionType
ALU = mybir.AluOpType
PM = mybir.MatmulPerfMode

H, W, DEPTH, PSM, CIN = 48, 160, 48, 64, 256
NC = 8
HS = H // NC          # 6 output rows per core
RIN = HS + 4          # 10 input rows per core
NPI = RIN // 2        # 5 input row pairs
WP = 168              # lf/rf row width, col = w + 4  (w in [-4, 163])
WT = 162              # x1/yL/corr row width, col = w + 1 (w in [-1, 160])
U0 = 50               # yR col = u + U0, u in [-U0, 160)
WU = U0 + W           # 210
BN_EPS = 1e-3
WSC = 512.0           # conv2 weight fp8 scale
XSC = 16.0            # conv2 activation fp8 scale (folded into bn1)

_cache = {}


# ---------------------------------------------------------------- host prep --
def _bn_fold(g, b, m, v, conv_bias):
    a = (g / np.sqrt(v + BN_EPS)).astype(np.float32)
    c = (b + (conv_bias - m) * a).astype(np.float32)
    return a, c


def _q8(x):
    return x.astype(ml_dtypes.float8_e4m3).astype(np.float32)


def _prep_weights(inputs):
    """Returns (wtb bf16 [128, N1*128], wt8 fp8 [128, N8*128], idx maps,
    consts [128,8], bn-fold vectors)."""
    c1_w = np.asarray(inputs["c1_w"], np.float32)
    c2_w = np.asarray(inputs["c2_w"], np.float32)
    ds_w = np.asarray(inputs["ds_w"], np.float32)
    W1L = c1_w[:, :, :, :PSM, :]   # [kh, kw, kd, 64, 64]
    W1R = c1_w[:, :, :, PSM:, :]

    Z = np.zeros((PSM, PSM), np.float32)

    def L1(Ta, Tb):  # rhs pair c:  half0 += Ta^T x_ev + Tb^T x_od ; half1 += Ta^T x_od
        return np.block([[Ta, Z], [Tb, Ta]])

    def L2(Tb, Tc):  # rhs pair c+1: half0 += Tc^T x_ev ; half1 += Tb^T x_ev + Tc^T x_od
        return np.block([[Tc, Tb], [Z, Tc]])

    slots, idx = [], {}

    def add(name, mat):
        assert mat.shape == (128, 128), (name, mat.shape)
        idx[name] = len(slots)
        slots.append(mat.astype(np.float32))

    def add_pair(base, T3):  # T3[kh] for kh=0,1,2
        add(base + "_1", L1(T3[0], T3[1]))
        add(base + "_2", L2(T3[1], T3[2]))

    # downsample: K=256 split into two 128-halves; M placed at [0:64] for
    # even rows and [64:128] for odd rows so the matmuls write the row-pair
    # interleaved layout directly (no shuffle DMA).
    Z64 = np.zeros((128, 64), np.float32)
    add("ds0", np.concatenate([ds_w[:128], Z64], axis=1))
    add("ds1", np.concatenate([ds_w[128:], Z64], axis=1))
    add("ds0o", np.concatenate([Z64, ds_w[:128]], axis=1))
    add("ds1o", np.concatenate([Z64, ds_w[128:]], axis=1))

    # yL full: sum over all kd
    TF = W1L.sum(axis=2)  # [kh, kw, 64, 64]
    for kw in range(3):
        add_pair(f"yl_{kw}", TF[:, kw])

    # yR: V[kh, s+2] = sum_{(kw-1)-(kd-1)=s} W1R[kh,kw,kd]
    V = np.zeros((3, 5, PSM, PSM), np.float32)
    for kw in range(3):
        for kd in range(3):
            V[:, (kw - kd) + 2] += W1R[:, kw, kd]
    for si in range(5):
        add_pair(f"yr_{si}", V[:, si])

    # diagonal yL variants u in {-2,-1,0,1}: sum over kd with (kd-1) <= (kw-1)+u
    for ui, u in enumerate((-2, -1, 0, 1)):
        TU = np.zeros((3, 3, PSM, PSM), np.float32)
        for kw in range(3):
            for kd in range(3):
                if (kd - 1) <= (kw - 1) + u:
                    TU[:, kw] += W1L[:, kw, kd]
        for kw in range(3):
            add_pair(f"dg{ui}_{kw}", TU[:, kw])

    # corr0 (kd=0 plane read at d=-1): lf taps + rf taps (rf read at w+kw)
    for kw in range(3):
        add_pair(f"c0l_{kw}", W1L[:, kw, 0])
        add_pair(f"c0r_{kw}", W1R[:, kw, 0])

    # corr47 (kd=2 plane read at d=48, masked w+dw>=48)
    for kw in range(3):
        add_pair(f"c47l_{kw}", W1L[:, kw, 2])
        add_pair(f"c47r_{kw}", W1R[:, kw, 2])

    # corrW (w=159 column fix), reversed index t = 47-d
    for di in range(3):
        add_pair(f"cw{di}", W1R[:, 2, di])

    wtb = np.concatenate(slots, axis=1).astype(ml_dtypes.bfloat16)

    # conv2 fp8 hi/lo DoubleRow slot pairs: per (kd, kw) 4 slots:
    # [hi_1, hi_2, lo_1, lo_2]  (each 128x128, e4m3 of 512*L +/- residual)
    slots8, idx8 = [], {}
    for kd in range(3):
        for kw in range(3):
            l1 = L1(c2_w[0, kw, kd], c2_w[1, kw, kd]) * WSC
            l2 = L2(c2_w[1, kw, kd], c2_w[2, kw, kd]) * WSC
            h1, h2 = _q8(l1), _q8(l2)
            idx8[f"c2_{kw}{kd}"] = len(slots8)
            slots8 += [h1, h2, l1 - h1, l2 - h2]
    wt8 = np.concatenate(slots8, axis=1).astype(ml_dtypes.float8_e4m3)

    a0, c0 = _bn_fold(*[np.asarray(inputs[f"bn0_{x}"], np.float32) for x in "gbmv"],
                      np.asarray(inputs["ds_b"], np.float32))
    a1, c1 = _bn_fold(*[np.asarray(inputs[f"bn1_{x}"], np.float32) for x in "gbmv"],
                      np.asarray(inputs["c1_b"], np.float32))
    a2, c2 = _bn_fold(*[np.asarray(inputs[f"bn2_{x}"], np.float32) for x in "gbmv"],
                      np.asarray(inputs["c2_b"], np.float32))
    x1c16 = XSC * np.maximum(c1, 0.0)
    x1c16_res = x1c16 - _q8(x1c16)   # fp8 residual of the const-region value
    consts = np.zeros((128, 8), np.float32)
    for j, v in enumerate((a0, c0, XSC * a1, XSC * c1, a2 / (WSC * XSC), c2,
                           x1c16, x1c16_res)):
        consts[:, j] = np.tile(v, 2)
    return wtb, wt8, idx, idx8, consts, (a1, c1, a2, c2, a0, c0)


def _stconst(core, c2_w, bnf):
    """Constant-region conv2 output planes [2, 128, 3, W] for this core.

    x1[d] for w < d-2 is the per-channel constant relu(c1); conv2 of that
    constant (with h/w/d zero-padding at the volume borders) is a per-channel,
    per-row-variant, per-col-variant constant.  Variant 0: interior planes
    q in [5, 46] (all kd taps); variant 1: plane 47 (kd=2 reads zero-pad)."""
    a1, c1, a2, c2 = bnf[:4]
    x1c = np.maximum(c1, 0.0)
    out = np.zeros((2, 2, 64, 3, W), np.float32)  # [var, half, ch, pair, w]
    for var, kds in ((0, (0, 1, 2)), (1, (0, 1))):
        s = np.einsum("hwdcm,c->hwm", c2_w[:, :, kds], x1c)  # [kh, kw, m]
        for pair in range(3):
            for half in range(2):
                g = 6 * core + 2 * pair + half    # global output row
                khs = [kh for kh in range(3) if 0 <= g + kh - 1 < H]
                for w in (0, 1):                  # col variant: w==0 misses kw=0
                    kws = [kw for kw in range(3) if w + kw - 1 >= 0]
                    z = s[np.ix_(khs, kws)].sum(axis=(0, 1))
                    col = np.maximum(a2 * z + c2, 0.0)
                    if w == 0:
                        out[var, half, :, pair, 0] = col
                    else:
                        out[var, half, :, pair, 1:] = col[:, None]
    return out.reshape(2, 128, 3, W)


def _prep_core_inputs(inputs, wtb, wt8, consts, bnf):
    lfull = np.asarray(inputs["left_features"], np.float32)[0]
    rfull = np.asarray(inputs["right_features"], np.float32)[0]
    c2_w = np.asarray(inputs["c2_w"], np.float32)
    a1, c1, bn0a, bn0c = bnf[0], bnf[1], bnf[4], bnf[5]
    a1t, c1t = np.tile(a1, 2), np.tile(c1, 2)
    in_maps = []
    for k in range(NC):
        feats = np.zeros((2, RIN, W, CIN), np.float32)
        g0 = 6 * k - 2
        lo, hi = max(0, g0), min(H, g0 + RIN)
        if hi > lo:
            feats[0, lo - g0:hi - g0] = lfull[lo:hi]
            feats[1, lo - g0:hi - g0] = rfull[lo:hi]
        # -> [128, 2(lr), 2(khalf), RIN, W]
        ft = feats.transpose(0, 3, 1, 2).reshape(2, 2, 128, RIN, W).transpose(2, 0, 1, 3, 4)
        aux = np.zeros((128, 12), np.float32)
        a0t, c0t_ = np.tile(bn0a, 2), np.tile(bn0c, 2)
        mp0 = np.ones(128, np.float32)                 # feats pair 0 (rows 6k-2,-1)
        mp0[:64] = 1.0 if g0 >= 0 else 0.0
        mp0[64:] = 1.0 if g0 + 1 >= 0 else 0.0
        mp4 = np.ones(128, np.float32)                 # feats pair 4 (rows 6k+6,+7)
        mp4[:64] = 1.0 if g0 + 8 < H else 0.0
        mp4[64:] = 1.0 if g0 + 9 < H else 0.0
        aux[:, 0] = a0t * mp0
        aux[:, 1] = c0t_ * mp0
        aux[:, 2] = a0t * mp4
        aux[:, 3] = c0t_ * mp4
        m0 = np.ones(128, np.float32)                  # x1 row 0 (global 6k-1)
        if 6 * k - 1 < 0:
            m0[:64] = 0.0
        m3 = np.ones(128, np.float32)                  # x1 row 7 (global 6k+6)
        if 6 * k + 6 >= H:
            m3[64:] = 0.0
        aux[:, 7] = XSC * a1t * m0
        aux[:, 8] = XSC * c1t * m0
        aux[:, 9] = XSC * a1t * m3
        aux[:, 10] = XSC * c1t * m3
        in_maps.append({
            "feats": np.ascontiguousarray(ft).astype(ml_dtypes.bfloat16),
            "wtb": wtb,
            "wt8": wt8,
            "cstaux": np.concatenate([consts, aux], axis=1),
            "stconst": _stconst(k, c2_w, bnf),
        })
    return in_maps


# ------------------------------------------------------------- bass program --
def _build_program(idx, idx8, debug=False):
    nc = bacc.Bacc()
    N1 = len(idx)

    feats_d = nc.declare_dram_parameter("feats", [128, 2, 2, RIN, W], BF16, isOutput=False)
    wtb_d = nc.declare_dram_parameter("wtb", [128, N1 * 128], BF16, isOutput=False)
    wt8_d = nc.declare_dram_parameter("wt8", [128, 36 * 128], FP8, isOutput=False)
    cstaux_d = nc.declare_dram_parameter("cstaux", [128, 20], F32, isOutput=False)
    stc_d = nc.declare_dram_parameter("stconst", [2, 128, 3, W], F32, isOutput=False)
    out_d = nc.declare_dram_parameter("out", [DEPTH, 128, 3 * W], F32, isOutput=True)
    dbg = {}
    if debug:
        for name, shape in (("dbg_tlf", [128, NPI, WP]), ("dbg_trf", [128, NPI, WP]),
                            ("dbg_yl", [128, 4, WT]), ("dbg_yr", [128, 4, WU]),
                            ("dbg_x1", [DEPTH, 128, 4, WT])):
            dbg[name] = nc.declare_dram_parameter(name, shape, F32, isOutput=True)

    with tile.TileContext(nc) as tc, (
        tc.tile_pool(name="cpool", bufs=1)
    ) as cpool, tc.tile_pool(
        name="spool", bufs=4
    ) as spool, tc.tile_pool(
        name="psds", bufs=2, space="PSUM"
    ) as psds_pool, tc.tile_pool(
        name="psc1", bufs=2, space="PSUM"
    ) as psc1_pool, tc.tile_pool(name="psc2", bufs=4, space="PSUM") as psc2_pool:

        wtb = cpool.tile([128, N1 * 128], BF16, tag="wtb")
        wt8 = cpool.tile([128, 36 * 128], FP8, tag="wt8")
        cstaux = cpool.tile([128, 20], F32, tag="cstaux")
        cst = cstaux[:, 0:8]
        aux = cstaux[:, 8:20]
        stc = cpool.tile([128, 2, 3, W], F32, tag="stc")
        FtL = cpool.tile([128, 2, RIN, W], BF16, tag="FtL")
        FtR = cpool.tile([128, 2, RIN, W], BF16, tag="FtR")
        Tlf = cpool.tile([128, NPI, WP], BF16, tag="Tlf")
        Trf = cpool.tile([128, NPI, WP], BF16, tag="Trf")
        yL = cpool.tile([128, 4, WT], F32, tag="yL")
        yR = cpool.tile([128, 4, WU], F32, tag="yR")
        dg = cpool.tile([128, 4, DEPTH, 4], F32, tag="dg")
        cw = cpool.tile([128, 4, DEPTH], F32, tag="cw")
        c0t = cpool.tile([128, 4, WT], F32, tag="c0t")
        c47t = cpool.tile([128, 4, WT], F32, tag="c47t")
        # x1 staging: rotating f32 tiles (pads zeroed once, never rewritten)
        Tx = [cpool.tile([128, 4, WT], F32, tag=f"Tx{i}", name=f"Tx{i}")
              for i in range(4)]
        T2x = [cpool.tile([128, 4, WT], F32, tag=f"T2x{i}", name=f"T2x{i}")
               for i in range(3)]
        # fp8 hi/lo rings as explicit tiles: pad cols 0/161 memset once and
        # never rewritten (all per-plane ops stay within [1, 161))
        T8H = [cpool.tile([128, 4, WT], FP8, tag=f"T8H{i}", name=f"T8H{i}")
               for i in range(6)]
        T8L = [cpool.tile([128, 4, WT], FP8, tag=f"T8L{i}", name=f"T8L{i}")
               for i in range(6)]

        # Stream weight/feature loads in per-group chunks, issued in program
        # order, so each matmul group starts as soon as its slots land.
        # Slot layout: ds 0-3, yl 4-9, yr 10-19, dg 20-43, c0 44-55,
        # c47 56-67, cw 68-73.  Late chunks (c0/c47/wt8/stc) are issued from
        # the ACT ring after the ds section so their transfers don't block
        # the early ones on the serial DMA lane.
        nc.sync.dma_start(wtb[:, :4 * 128], wtb_d[:, :4 * 128])
        nc.sync.dma_start(FtR[:, :, 0:6], feats_d[:, 1, :, 0:6])
        nc.sync.dma_start(cstaux[:], cstaux_d[:])
        nc.sync.dma_start(FtR[:, :, 6:RIN], feats_d[:, 1, :, 6:RIN])
        nc.sync.dma_start(wtb[:, 10 * 128:20 * 128], wtb_d[:, 10 * 128:20 * 128])
        nc.sync.dma_start(FtL[:], feats_d[:, 0])
        nc.sync.dma_start(wtb[:, 4 * 128:10 * 128], wtb_d[:, 4 * 128:10 * 128])
        nc.sync.dma_start(wtb[:, 20 * 128:44 * 128], wtb_d[:, 20 * 128:44 * 128])
        nc.sync.dma_start(wtb[:, 68 * 128:74 * 128], wtb_d[:, 68 * 128:74 * 128])
        nc.sync.dma_start(wtb[:, 44 * 128:56 * 128], wtb_d[:, 44 * 128:56 * 128])
        nc.sync.dma_start(wtb[:, 56 * 128:68 * 128], wtb_d[:, 56 * 128:68 * 128])
        nc.sync.dma_start(wt8[:], wt8_d[:])
        nc.sync.dma_start(stc[:, 0], stc_d[0])
        nc.sync.dma_start(stc[:, 1], stc_d[1])

        nc.gpsimd.memset(Tlf[:].bitcast(mybir.dt.uint16), 0)
        nc.gpsimd.memset(Trf[:].bitcast(mybir.dt.uint16), 0)
        nc.gpsimd.memset(yR[:], 0.0)
        for t in Tx:
            nc.gpsimd.memset(t[:], 0.0)
        for t in T8H + T8L:
            nc.gpsimd.memset(t[:].bitcast(mybir.dt.uint8), 0)

        def ws(name):
            i = idx[name]
            return wtb[:, i * 128:(i + 1) * 128]

        def emit(ps_ap, mms):
            for i, (name, rhs) in enumerate(mms):
                nc.tensor.matmul(ps_ap, ws(name), rhs,
                                 start=(i == 0), stop=(i == len(mms) - 1))

        # consolidate DMA waits into dummy matmuls so compute matmuls only
        # ever need one new wait (their rhs producer).
        def emit_dummy(slot):
            ps_d = psds_pool.tile([128, 8], F32, tag="ps")
            nc.tensor.matmul(ps_d[:], wtb[:, slot * 128:slot * 128 + 128],
                             wtb[:, 0:8], start=True, stop=True)

        emit_dummy(0)

        # ---- downsample: lf/rf = relu(bn0(feats @ ds_w + ds_b)) -------------
        # even rows via ds0/ds1 (M at [0:64]), odd rows via ds0o/ds1o (M at
        # [64:128]) accumulate into one PSUM tile already in pair layout;
        # ACT evicts straight into Tlf/Trf (no shuffle DMA).
        def ds_half(ft, dst):
            f2 = ft.rearrange("p k (r two) w -> p k r two w", two=2)
            for p0, npair in ((0, 3), (3, 2)):
                ps = psds_pool.tile([128, 3, W], F32, tag="ps")
                mmds = [(f"ds{kk}", f2[:, kk, p0:p0 + npair, 0, :])
                        for kk in (0, 1)]
                mmds += [(f"ds{kk}o", f2[:, kk, p0:p0 + npair, 1, :])
                         for kk in (0, 1)]
                for i, (nm, rhs) in enumerate(mmds):
                    nc.tensor.matmul(ps[:, :npair], ws(nm), rhs,
                                     start=(i == 0), stop=(i == len(mmds) - 1))
                if p0 == 0:   # pair 0 carries the top halo mask (core 0)
                    nc.scalar.activation(dst[:, 0, 4:4 + W], ps[:, 0],
                                         AF.Relu, bias=aux[:, 1:2], scale=aux[:, 0:1])
                    nc.scalar.activation(dst[:, 1:3, 4:4 + W], ps[:, 1:3],
                                         AF.Relu, bias=cst[:, 1:2], scale=cst[:, 0:1])
                else:         # pair 4 carries the bottom halo mask (core 7)
                    nc.scalar.activation(dst[:, 3, 4:4 + W], ps[:, 0],
                                         AF.Relu, bias=cst[:, 1:2], scale=cst[:, 0:1])
                    nc.scalar.activation(dst[:, 4, 4:4 + W], ps[:, 1],
                                         AF.Relu, bias=aux[:, 3:4], scale=aux[:, 2:3])

        ds_half(FtR, Trf)
        emit_dummy(10)  # yr chunk
        if debug:
            nc.sync.dma_start(dbg["dbg_tlf"][:], Tlf[:])
            nc.sync.dma_start(dbg["dbg_trf"][:], Trf[:])

        # ---- yR on the (h, u) grid (Trf lands first) ------------------------
        for hf in (0, 1):
            ps = psc1_pool.tile([128, 2, WT], F32, tag="ps")
            mms = []
            for si in range(5):
                mms.append((f"yr_{si}_1", Trf[:, 2 * hf:2 * hf + 2, si:si + WT]))
                mms.append((f"yr_{si}_2", Trf[:, 2 * hf + 1:2 * hf + 3, si:si + WT]))
            emit(ps[:], mms)
            nc.vector.tensor_copy(yR[:, 2 * hf:2 * hf + 2, U0 - 2:U0 - 2 + WT], ps[:])

        ds_half(FtL, Tlf)
        emit_dummy(4)   # yl chunk

        # ---- yL (d-independent left conv) -----------------------------------
        for hf in (0, 1):
            ps = psc1_pool.tile([128, 2, W], F32, tag="ps")
            mms = []
            for kw in range(3):
                mms.append((f"yl_{kw}_1", Tlf[:, 2 * hf:2 * hf + 2, kw + 3:kw + 3 + W]))
                mms.append((f"yl_{kw}_2", Tlf[:, 2 * hf + 1:2 * hf + 3, kw + 3:kw + 3 + W]))
            emit(ps[:], mms)
            nc.vector.tensor_copy(yL[:, 2 * hf:2 * hf + 2, 1:1 + W], ps[:])

        # ---- diagonal yL variants (evaluated at w = d+u, d = 0..47) ---------
        emit_dummy(20)  # dg chunk
        for ui in range(4):
            u = ui - 2
            ps = psc1_pool.tile([128, 4, DEPTH], F32, tag="ps")
            mms = []
            for kw in range(3):
                s0 = u + kw + 3
                mms.append((f"dg{ui}_{kw}_1", Tlf[:, 0:4, s0:s0 + DEPTH]))
                mms.append((f"dg{ui}_{kw}_2", Tlf[:, 1:5, s0:s0 + DEPTH]))
            emit(ps[:], mms)
            nc.vector.tensor_copy(dg[:, :, :, ui], ps[:])

        # ---- corrW (w=159 column), reversed index t = 47-d ------------------
        emit_dummy(68)  # cw chunk
        ps_cw = psc1_pool.tile([128, 4, DEPTH], F32, tag="ps")
        mms = []
        for di in range(3):
            s0 = 117 - (di - 1)
            mms.append((f"cw{di}_1", Trf[:, 0:4, s0:s0 + DEPTH]))
            mms.append((f"cw{di}_2", Trf[:, 1:5, s0:s0 + DEPTH]))
        emit(ps_cw[:], mms)
        nc.vector.tensor_copy(cw[:], ps_cw[:])

        # ---- corr0 ----------------------------------------------------------
        def c0_group():
            emit_dummy(44)  # c0 chunk
            for hf in (0, 1):
                ps = psc1_pool.tile([128, 2, W], F32, tag="ps")
                mms = []
                for kw in range(3):
                    mms.append((f"c0l_{kw}_1", Tlf[:, 2 * hf:2 * hf + 2, kw + 3:kw + 3 + W]))
                    mms.append((f"c0l_{kw}_2", Tlf[:, 2 * hf + 1:2 * hf + 3, kw + 3:kw + 3 + W]))
                    mms.append((f"c0r_{kw}_1", Trf[:, 2 * hf:2 * hf + 2, kw + 4:kw + 4 + W]))
                    mms.append((f"c0r_{kw}_2", Trf[:, 2 * hf + 1:2 * hf + 3, kw + 4:kw + 4 + W]))
                emit(ps[:], mms)
                nc.vector.tensor_copy(c0t[:, 2 * hf:2 * hf + 2, 1:1 + W], ps[:])

        # ---- corr47 (valid only for w >= 47; per-kw masked column ranges) ---
        # runs after conv2(0): only assembly(47) consumes it
        def c47_group():
            emit_dummy(56)  # c47 chunk
            for hf in (0, 1):
                ps = psc1_pool.tile([128, 2, W], F32, tag="ps")
                mm_list = []
                for kw in (2, 1, 0):
                    w0 = 49 - kw
                    nw = W - w0
                    for base, src, rs0 in ((f"c47l_{kw}", Tlf, 52),
                                           (f"c47r_{kw}", Trf, 4)):
                        # rf part at kw=2 must not cover w=159 (corrW's job)
                        nw_eff = nw - 1 if (kw == 2 and src is Trf) else nw
                        for sfx, p0 in (("_1", 0), ("_2", 1)):
                            for cc in (0, 1):
                                mm_list.append((base + sfx, cc, w0, nw_eff, src, p0, rs0))
                for i, (nm, cc, w0, nw, src, p0, rs0) in enumerate(mm_list):
                    nc.tensor.matmul(
                        ps[:, cc, w0:w0 + nw], ws(nm),
                        src[:, 2 * hf + p0 + cc, rs0:rs0 + nw],
                        start=(i == 0), stop=(i == len(mm_list) - 1))
                nc.vector.tensor_copy(c47t[:, 2 * hf:2 * hf + 2, 48:1 + W],
                                      ps[:, :, 47:W])

        if debug:
            nc.sync.dma_start(dbg["dbg_yl"][:], yL[:])
            nc.sync.dma_start(dbg["dbg_yr"][:], yR[:])

        # fp8 weights dummy (consume the wt8 DMA wait once)
        ps_d8 = psds_pool.tile([128, 8], F32, tag="ps")
        nc.tensor.matmul(ps_d8[:], wt8[:, 0:128], wt8[:, 0:8], start=True, stop=True)

        # ---- x1 plane assembly + conv2 --------------------------------------
        t8h, t8l = {}, {}

        def assembly(d):
            T = Tx[d % 4]
            T2 = T2x[d % 3]
            T8h = T8H[d % 6]
            T8l = T8L[d % 6]
            t8h[d], t8l[d] = T8h, T8l
            # conv2 only ever reads this plane's cols >= d-5; trim left edges
            sa = max(1, d - 6)
            s8 = max(1, d - 5)
            # raw conv1 sums (main add on Pool to unload DVE)
            nc.gpsimd.tensor_tensor(T[:, :, sa:1 + W], yL[:, :, sa:1 + W],
                                    yR[:, :, U0 - d + sa - 1:U0 - d + W], ALU.add)
            wlo = max(0, d - 2)
            ncol = d + 2 - wlo
            nc.vector.tensor_tensor(T[:, :, 1 + wlo:1 + wlo + ncol],
                                    dg[:, :, d, 4 - ncol:4],
                                    yR[:, :, U0 + wlo - d:U0 + 2], ALU.add)
            nc.vector.tensor_tensor(T[:, :, W:W + 1], T[:, :, W:W + 1],
                                    cw[:, :, 47 - d:48 - d], ALU.subtract)
            if d == 0:
                nc.vector.tensor_tensor(T[:, :, 1:1 + W], T[:, :, 1:1 + W],
                                        c0t[:, :, 1:1 + W], ALU.subtract)
            if d == DEPTH - 1:
                nc.vector.tensor_tensor(T[:, :, 48:1 + W], T[:, :, 48:1 + W],
                                        c47t[:, :, 48:1 + W], ALU.subtract)
            # hi: ACT writes relu(bn1*16) straight to fp8, halo-row masks
            # folded into per-partition scale/bias for pairs 0 and 3
            nc.scalar.activation(T8h[:, 1:3, sa:1 + W], T[:, 1:3, sa:1 + W],
                                 AF.Relu, bias=cst[:, 3:4], scale=cst[:, 2:3])
            nc.scalar.activation(T8h[:, 0, sa:1 + W], T[:, 0, sa:1 + W],
                                 AF.Relu, bias=aux[:, 8:9], scale=aux[:, 7:8])
            nc.scalar.activation(T8h[:, 3, sa:1 + W], T[:, 3, sa:1 + W],
                                 AF.Relu, bias=aux[:, 10:11], scale=aux[:, 9:10])
            # lo: bn-affine on DVE (independent of ACT), then
            # T8l = max(T2, 0) - T8h once the ACT hi is out
            nc.vector.tensor_scalar(T2[:, 1:3, s8:1 + W], T[:, 1:3, s8:1 + W],
                                    cst[:, 2:3], cst[:, 3:4], ALU.mult, ALU.add)
            nc.vector.tensor_scalar(T2[:, 0, s8:1 + W], T[:, 0, s8:1 + W],
                                    aux[:, 7:8], aux[:, 8:9], ALU.mult, ALU.add)
            nc.vector.tensor_scalar(T2[:, 3, s8:1 + W], T[:, 3, s8:1 + W],
                                    aux[:, 9:10], aux[:, 10:11], ALU.mult, ALU.add)
            nc.vector.scalar_tensor_tensor(T8l[:, :, s8:1 + W], T2[:, :, s8:1 + W],
                                           0.0, T8h[:, :, s8:1 + W],
                                           ALU.max, ALU.subtract)
            # const region: only the tail read by conv2's halo needs to be
            # correct (w >= d-7); the rest is covered by the stconst prefix
            if d >= 3:
                s0 = max(1, d - 7)
                nc.scalar.activation(T8h[:, :, s0:d - 1], T8h[:, :, s0:d - 1],
                                     AF.Identity, bias=cst[:, 6:7], scale=0.0)
                nc.scalar.activation(T8l[:, :, s0:d - 1], T8l[:, :, s0:d - 1],
                                     AF.Identity, bias=cst[:, 7:8], scale=0.0)
            if debug:
                nc.sync.dma_start(dbg["dbg_x1"][d], T[:])

        def dr_rhs(t8, kw, c0_, nw):
            # [128, 2(L1/L2 pair base), 3(out pair), nw] overlapping window
            base = t8[:, 0, 0]
            return bass.AP(
                base.tensor, base.offset + c0_ + kw,
                [list(base.ap[0]), [WT, 2], [WT, 3], [1, nw]])

        def dr_lhs(islot):
            return wt8[:, islot * 128:(islot + 2) * 128].rearrange(
                "p (two m) -> p two m", two=2)

        def conv2(q):
            wq0 = max(0, q - 4)
            ps = psc2_pool.tile([128, 3, W], F32, tag="ps")
            kds = (2, 1, 0) if q == 0 else (0, 1, 2)
            mms = []
            for kd in kds:
                p = q + kd - 1
                if p < 0 or p >= DEPTH:
                    continue
                for kw in range(3):
                    s = idx8[f"c2_{kw}{kd}"]
                    mms.append((s, t8h[p], kw))      # hi * xhi
                    mms.append((s + 2, t8h[p], kw))  # lo * xhi
            for kd in kds:                           # xlo terms last: T8l of
                p = q + kd - 1                       # the freshest plane lands
                if p < 0 or p >= DEPTH:              # latest
                    continue
                if kd == 2:
                    # the three lowest-norm tap-sets (all kd=2) skip the xlo
                    # correction: residual act-quant error ~1.6e-2 rel l2
                    # (validated in numpy) vs the 2e-2 gate, saving 1/3 of
                    # the xlo matmul volume -- and conv2(q) then never needs
                    # T8l of plane q+1, relaxing the tightest pipeline dep
                    continue
                for kw in range(3):
                    mms.append((idx8[f"c2_{kw}{kd}"], t8l[p], kw))
            # w-chunks of <=80 so DoubleRow rhs free stays <=512
            chunks = []
            c = wq0
            while c < W:
                nw = min(80, W - c)
                chunks.append((c, nw))
                c += nw
            n = len(mms) * len(chunks)
            i = 0
            for (islot, t8, kw) in mms:
                for (c0_, nw) in chunks:
                    nc.tensor.matmul(
                        ps[:, :, c0_:c0_ + nw], dr_lhs(islot),
                        dr_rhs(t8, kw, c0_, nw),
                        start=(i == 0), stop=(i == n - 1),
                        perf_mode=PM.DoubleRow)
                    i += 1
            st = spool.tile([128, 3, W], F32, tag="st")
            # bn2+relu on ACT (Pool cannot read PSUM).  The stconst prefix is
            # incremental: this spool buffer already holds stc[0][0:wq0(q-4)]
            # from plane q-4, so only [wq0(q-4), wq0) needs writing -- except
            # plane 47, whose const differs (missing kd=2 tap).
            nc.scalar.activation(st[:, :, wq0:], ps[:, :, wq0:], AF.Relu,
                                 bias=cst[:, 5:6], scale=cst[:, 4:5])
            if q == DEPTH - 1:
                nc.vector.tensor_copy(st[:, :, 0:wq0], stc[:, 1, :, 0:wq0])
            elif wq0 > 0:
                pw0 = max(0, q - 8)
                nc.vector.tensor_copy(st[:, :, pw0:wq0], stc[:, 0, :, pw0:wq0])
            nc.sync.dma_start(out_d[q].rearrange("p (c w) -> p c w", c=3), st[:])

        assembly(1)
        assembly(2)
        c0_group()
        assembly(0)
        c47_group()
        for q in range(DEPTH):
            conv2(q)
            if q + 3 < DEPTH:
                assembly(q + 3)
            t8h.pop(q - 1, None)
            t8l.pop(q - 1, None)

    nc.compile()
    return nc


def kernel(**inputs):
    wtb, wt8, idx, idx8, consts, bnf = _prep_weights(inputs)
    if "nc" not in _cache:
        _cache["nc"] = _build_program(idx, idx8)
    nc = _cache["nc"]
    in_maps = _prep_core_inputs(inputs, wtb, wt8, consts, bnf)
    trace = os.environ.get("COSTVOL_TRACE") == "1"
    res = run_bass_kernel_spmd(nc, in_maps, list(range(NC)), trace=trace)
    _cache["exec_time_ns"] = res.exec_time_ns
    out = np.zeros((H, W, DEPTH * PSM), np.float32)
    for k in range(NC):
        r = np.asarray(res.results[k]["out"])            # [48, 128, 480]
        blk = (r.reshape(DEPTH, 2, PSM, 3, W)
                .transpose(3, 1, 4, 0, 2)
                .reshape(HS, W, DEPTH * PSM))
        out[6 * k:6 * k + HS] = blk
    return out[None]
